# revision 14
# baseline (speedup 1.0000x reference)
"""Trainium2 Bass kernel for nn_CF_68169720922624 (segment_reduce CF predictor).

Computation (see reference):
    ub[u]   = masked mean of rating_mtx[u, :] over nonzero entries
    score[b]= sum_u  S[user[b], u] * (R[u, item[b]] - ub[u])
    out[b]  = sigmoid(score[b] + user_bias[user[b]] + item_bias[item[b]] + gb) * 5

Sharding v5 (batch-sharded phase B, user-sharded phase A):
  Core k owns queries b in [1024k, 1024k+1024). The host pre-gathers the
  S-rows and R-columns those queries touch into dense fp16 tables with the
  contraction dim (users u) on rows, 4 row-blocks packed per DRAM row so
  every DMA descriptor moves 8 KB per partition:
    et [2048, 4096] fp16 : row 128m+p holds S[user[q0+j], 128(4m+b)+p]
                           for b in 0..3 (4 KB each)         [16 macro tiles]
    ft [2048, 4096] fp16 : same layout for R[u, item[q0+j]]
  Pad user-rows 8060..8062 carry the bias terms (E row = 1, F row = gb /
  item_bias[item] / user_bias[user]), so the contraction accumulates the
  bias for free.  Phase A input is the transposed fp8 ratings stream
  (item-rows on partitions), 4 blocks packed per row: r8 [2048, 8192].

All reduction matmuls here have M=1, which would waste 127/128 of the PE
array - so they are issued as CONCURRENT col-tiled matmuls
(tile_position=(0, 32j)): phase A packs s_h0/s_h1/c_h0/c_h1 into one PSUM
bank at partitions 0/32/64/96 (4 matmuls run in ~the span of one); phase B
packs p_h0/p_h1 at partitions 0/32.

Per core:
  Phase A (16 macro tiles of 4 blocks):
    s += ones^T @ r8; nnz masks as fp8 0/1 via ACT Sign / DVE is_ne
    (split across engines); c += ones^T @ mask.  All matmuls fp8 N=512.
  AllGather of RAW (s | c) as fp16 (4 KB per core -> 32 KB); the
    masked-mean division happens after the collective, directly in the
    final transposed layout: scT [128, 8, 2, 8] via one rearranged DMA;
    ubTg = s / max(c, 1)  (user 128(8g+t)+p at free index (g, t)).
  Phase B (16 macro tiles): fpt = ft - ub (split ACT bias-activation /
    DVE tensor_scalar), ppt = et * fpt (DVE, some blocks on GpSimd),
    p += ones^T @ ppt (2 col-tiled fp16 matmuls per block, accumulated
    over all 64 blocks).  No prefetch: the r8 stream owns the DMA queues
    first; et/ft stream DMAs are issued after the ub chain and fill the
    queues for the rest of the kernel (pools sized so the stream never
    stalls while the collective is in flight).
  Tail: sigmoid * 5 straight out of PSUM rows 0/32, two DMAs out.
"""

import numpy as np
import ml_dtypes
from contextlib import ExitStack

import concourse.bass as bass
import concourse.bacc as bacc
import concourse.tile as tile
from concourse import mybir
from concourse.bass_utils import run_bass_kernel_spmd

F32 = mybir.dt.float32
F16 = mybir.dt.float16
F8 = mybir.dt.float8e4
NPF16 = np.float16
NPF8 = ml_dtypes.float8_e4m3

NCORES = 8
U = 8001
I = 16001
B = 8192
UPC = 1024          # users per core for phase A (last core: 833 real)
BPC = 1024          # queries per core
MAC = 16            # macro tiles (4 blocks each) in both phases
NBLK = 64           # row blocks of 128 (63 real + 1 pad)
IP2 = 16384         # padded item rows for phase A (64 * 256)
UG2 = 8192          # padded user rows for phase B (64 * 128)

E_BUFS = 7
F_BUFS = 7
R8_BUFS = 12
MASK_DVE = {0, 1, 2, 4, 5}   # t % 8 values whose masks run on DVE (rest ACT)
SUB_DVE_MOD = 3     # of every 10 blocks, this many subtracts on DVE (rest ACT)
MULT_GP_MOD = 5     # every MULT_GP_MOD-th block's multiply runs on GpSimd

# bias rows hidden in the user padding (8001..8063)
ROW_GB = 8060
ROW_IB = 8061
ROW_UB = 8062

_CACHED = {}


def build_program():
    nc = bacc.Bacc(num_devices=NCORES, num_swdge_queues=4)

    r8 = nc.dram_tensor("r8", [MAC * 128, 8192], F8, kind="ExternalInput")
    et_d = nc.dram_tensor("et", [MAC * 128, 4096], F16, kind="ExternalInput")
    ft_d = nc.dram_tensor("ft", [MAC * 128, 4096], F16, kind="ExternalInput")
    out = nc.dram_tensor("out", [1, BPC], F32, kind="ExternalOutput")

    with ExitStack() as ctx:
        tc = ctx.enter_context(tile.TileContext(nc))
        singles = ctx.enter_context(tc.tile_pool(name="singles", bufs=1))
        r8_pool = ctx.enter_context(tc.tile_pool(name="r8_pool", bufs=R8_BUFS))
        m_pool = ctx.enter_context(tc.tile_pool(name="m_pool", bufs=6))
        e_pool = ctx.enter_context(tc.tile_pool(name="e_pool", bufs=E_BUFS))
        f_pool = ctx.enter_context(tc.tile_pool(name="f_pool", bufs=F_BUFS))
        fp_pool = ctx.enter_context(tc.tile_pool(name="fp_pool", bufs=4))
        pp_pool = ctx.enter_context(tc.tile_pool(name="pp_pool", bufs=5))
        psA = ctx.enter_context(tc.tile_pool(name="psA", bufs=1, space="PSUM"))
        psB = ctx.enter_context(tc.tile_pool(name="psB", bufs=1, space="PSUM"))
        dram = ctx.enter_context(tc.tile_pool(name="dram", bufs=1, space="DRAM"))

        ones8 = singles.tile([128, 16], F8)
        nc.vector.memset(ones8, 1.0)
        ones16 = singles.tile([128, 1], F16)
        nc.vector.memset(ones16, 1.0)

        # ---- Phase A: s/c accumulate in ONE PSUM bank at partitions
        # 0 (s_h0), 32 (s_h1), 64 (c_h0), 96 (c_h1) via col-tiling.
        ps_a = psA.tile([128, 512], F32)

        for m in range(MAC):
            for b in range(4):
                t = 4 * m + b
                r8t = r8_pool.tile([128, 2, 1024], F8, name="r8t")
                nc.sync.dma_start(
                    r8t, r8[m * 128:(m + 1) * 128,
                            2048 * b:2048 * (b + 1)].rearrange(
                        "p (i n) -> p i n", i=2)
                )
                m8 = m_pool.tile([128, 2, 1024], F8, name="m8")
                if t % 8 in MASK_DVE:
                    nc.vector.tensor_scalar(
                        m8, r8t[:, :, :], 0.0, None,
                        mybir.AluOpType.not_equal,
                    )
                else:
                    nc.scalar.activation(
                        m8, r8t[:, :, :],
                        mybir.ActivationFunctionType.Sign,
                    )
                # 4 concurrent col-tiled matmuls per (block, i): s_h0, s_h1,
                # c_h0, c_h1 at PSUM partitions 0/32/64/96.
                for i in range(2):
                    st = (t == 0 and i == 0)
                    sp = (t == NBLK - 1 and i == 1)
                    for h in range(2):
                        nc.tensor.matmul(
                            ps_a[32 * h:32 * h + 1, :],
                            ones8[:, 0:1],
                            r8t[:, i, h * 512:(h + 1) * 512],
                            start=st, stop=sp,
                            tile_position=(0, 32 * h),
                        )
                        nc.tensor.matmul(
                            ps_a[64 + 32 * h:64 + 32 * h + 1, :],
                            ones8[:, 0:1],
                            m8[:, i, h * 512:(h + 1) * 512],
                            start=st, stop=sp,
                            tile_position=(0, 64 + 32 * h),
                        )

        # ---- stage (s | c) to fp16 and AllGather: 4 KB/core -> 32 KB.
        sc_sb = singles.tile([128, 512], F16)
        nc.vector.tensor_copy(sc_sb[0:1, :], ps_a[0:1, :])
        nc.vector.tensor_copy(sc_sb[32:33, :], ps_a[32:33, :])
        nc.scalar.copy(sc_sb[64:65, :], ps_a[64:65, :])
        nc.scalar.copy(sc_sb[96:97, :], ps_a[96:97, :])
        sc_dram = dram.tile([1, 2048], F16, name="sc_dram")
        for j in range(4):
            nc.sync.dma_start(
                sc_dram[0:1, j * 512:(j + 1) * 512], sc_sb[32 * j:32 * j + 1, :]
            )
        cc_out = dram.tile([1, NCORES * 2048], F16, name="cco")
        nc.gpsimd.collective_compute(
            "AllGather",
            mybir.AluOpType.bypass,
            replica_groups=[list(range(NCORES))],
            ins=[sc_dram.opt()],
            outs=[cc_out.opt()],
        )
        # scT[p, g, h, t] = (s|c)[core g][half h][128t + p];
        # global user-block T = 8g + t lives at free index (g, t).
        scT = singles.tile([128, NCORES, 2, 8], F16)
        nc.sync.dma_start(
            scT.rearrange("p g h t -> p (g h t)"),
            cc_out[0:1, :].rearrange("o (j p) -> (o j) p", j=128, p=128),
            transpose=True,
        )
        cmaxT = singles.tile([128, NCORES, 8], F32)
        nc.vector.tensor_scalar_max(cmaxT, scT[:, :, 1, :], 1.0)
        crecT = singles.tile([128, NCORES, 8], F32)
        nc.vector.reciprocal(crecT, cmaxT)
        ubTg = singles.tile([128, NCORES, 8], F32)
        nc.vector.tensor_tensor(
            ubTg, scT[:, :, 0, :], crecT, mybir.AluOpType.mult
        )
        negub = singles.tile([128, NCORES, 8], F32)
        nc.vector.tensor_scalar_mul(negub, ubTg, -1.0)

        # ---- phase-B stream DMAs (issued after the ub chain so its small
        # DMAs are not queued behind the 33 MB bulk stream).
        e_tiles = {}
        f_tiles = {}
        for m in range(MAC):
            et = e_pool.tile([128, 4, BPC], F16, name="et_m")
            nc.sync.dma_start(
                et, et_d[m * 128:(m + 1) * 128, :].rearrange(
                    "p (b n) -> p b n", b=4)
            )
            ft = f_pool.tile([128, 4, BPC], F16, name="ft_m")
            nc.sync.dma_start(
                ft, ft_d[m * 128:(m + 1) * 128, :].rearrange(
                    "p (b n) -> p b n", b=4)
            )
            e_tiles[m] = et
            f_tiles[m] = ft

        # ---- Phase B: p[b] += sum_u et[u, b] * (ft[u, b] - ub[u]).
        # p_h0 accumulates at PSUM partition 0, p_h1 at partition 32.
        ps_b = psB.tile([128, 512], F32)
        for m in range(MAC):
            et, ft = e_tiles[m], f_tiles[m]
            for b in range(4):
                t = 4 * m + b
                g, tt = t // 8, t % 8
                fpt = fp_pool.tile([128, BPC], F16)
                if t >= NBLK - 4 or t % 12 < 5:
                    nc.vector.tensor_scalar(
                        fpt, ft[:, b, :], ubTg[:, g, tt:tt + 1], None,
                        mybir.AluOpType.subtract,
                    )
                else:
                    nc.scalar.activation(
                        fpt, ft[:, b, :],
                        mybir.ActivationFunctionType.Identity,
                        bias=negub[:, g, tt:tt + 1],
                    )
                ppt = pp_pool.tile([128, BPC], F16)
                if t % 4 == 2 and t < NBLK - 4:
                    nc.gpsimd.tensor_tensor(
                        ppt, et[:, b, :], fpt, mybir.AluOpType.mult
                    )
                else:
                    nc.vector.tensor_tensor(
                        ppt, et[:, b, :], fpt, mybir.AluOpType.mult
                    )
                for h in range(2):
                    nc.tensor.matmul(
                        ps_b[32 * h:32 * h + 1, :],
                        ones16[:, :],
                        ppt[:, h * 512:(h + 1) * 512],
                        start=(t == 0), stop=(t == NBLK - 1),
                        tile_position=(0, 32 * h),
                    )

        # ---- Tail: sigmoid * 5 straight out of PSUM rows 0 / 32.
        pred = singles.tile([128, 512], F32)
        for h in range(2):
            r = 32 * h
            nc.scalar.activation(
                pred[r:r + 1, :], ps_b[r:r + 1, :],
                mybir.ActivationFunctionType.Sigmoid,
            )
            nc.vector.tensor_scalar_mul(
                pred[r:r + 1, :], pred[r:r + 1, :], 5.0
            )
            nc.sync.dma_start(
                out[0:1, h * 512:(h + 1) * 512], pred[r:r + 1, :]
            )

    nc.finalize()
    return nc


def _pack4(a):
    """[64*128, W] row-blocks -> [16*128, 4*W] macro rows."""
    W = a.shape[1]
    return np.ascontiguousarray(
        a.reshape(MAC, 4, 128, W).transpose(0, 2, 1, 3).reshape(MAC * 128, 4 * W)
    )


def prepare_inputs(user, item, rating_mtx, user_similarity, user_bias,
                   item_bias, global_bias):
    user = np.asarray(user).astype(np.int64)
    item = np.asarray(item).astype(np.int64)
    R = np.asarray(rating_mtx, dtype=np.float32)
    S = np.asarray(user_similarity, dtype=np.float32)
    ubias = np.asarray(user_bias, dtype=np.float32)
    ibias = np.asarray(item_bias, dtype=np.float32)
    gb = np.float32(np.asarray(global_bias))

    R16 = R.astype(NPF16)
    Rt16 = np.ascontiguousarray(R16.T)       # [I, U] for fast column gather
    S16 = S.astype(NPF16)

    in_maps = []
    for k in range(NCORES):
        # ---- phase A: fp8 transposed ratings for this core's user slice,
        # interleaved in (i, p) row pairs per 256-item block.
        u_lo = k * UPC
        u_hi = min(u_lo + UPC, U)
        nu = u_hi - u_lo
        r8full = np.zeros((IP2, UPC), NPF8)
        r8full[:I, :nu] = R[u_lo:u_hi, :].T.astype(NPF8)
        r8i = r8full.reshape(NBLK, 2, 128, UPC).transpose(0, 2, 1, 3).reshape(
            NBLK * 128, 2 * UPC)
        r8 = _pack4(r8i)

        # ---- phase B: dense gathered tables for this core's query slice.
        js = slice(k * BPC, (k + 1) * BPC)
        uk = user[js]
        ik = item[js]
        et = np.zeros((UG2, BPC), NPF16)
        et[:U, :] = np.ascontiguousarray(S16[uk, :].T)
        ft = np.zeros((UG2, BPC), NPF16)
        ft[:U, :] = np.ascontiguousarray(Rt16[ik, :].T)
        # bias terms as extra "users" in the padding
        et[ROW_GB:ROW_UB + 1, :] = NPF16(1.0)
        ft[ROW_GB, :] = NPF16(gb)
        ft[ROW_IB, :] = ibias[ik].astype(NPF16)
        ft[ROW_UB, :] = ubias[uk].astype(NPF16)

        in_maps.append({"r8": r8, "et": _pack4(et), "ft": _pack4(ft)})
    return in_maps


def kernel(user, item, rating_mtx, user_similarity, user_bias, item_bias,
           global_bias, _trace=False):
    if "nc" not in _CACHED:
        _CACHED["nc"] = build_program()
    nc = _CACHED["nc"]

    in_maps = prepare_inputs(
        user, item, rating_mtx, user_similarity, user_bias, item_bias,
        global_bias,
    )
    res = run_bass_kernel_spmd(nc, in_maps, core_ids=list(range(NCORES)))
    if _trace:
        # cold traced runs have hung; trace only after a warm run
        res = run_bass_kernel_spmd(
            nc, in_maps, core_ids=list(range(NCORES)), trace=True
        )
    _CACHED["last_results"] = res

    out = np.concatenate(
        [np.asarray(res.results[k]["out"]).reshape(-1) for k in range(NCORES)]
    )
    return out.astype(np.float32)


# revision 16
# speedup vs baseline: 1.0235x; 1.0235x over previous
"""Trainium2 Bass kernel for nn_CF_68169720922624 (segment_reduce CF predictor).

Computation (see reference):
    ub[u]   = masked mean of rating_mtx[u, :] over nonzero entries
    score[b]= sum_u  S[user[b], u] * (R[u, item[b]] - ub[u])
    out[b]  = sigmoid(score[b] + user_bias[user[b]] + item_bias[item[b]] + gb) * 5

Sharding v5 (batch-sharded phase B, user-sharded phase A):
  Core k owns queries b in [1024k, 1024k+1024). The host pre-gathers the
  S-rows and R-columns those queries touch into dense fp16 tables with the
  contraction dim (users u) on rows, 4 row-blocks packed per DRAM row so
  every DMA descriptor moves 8 KB per partition:
    et [2048, 4096] fp16 : row 128m+p holds S[user[q0+j], 128(4m+b)+p]
                           for b in 0..3 (4 KB each)         [16 macro tiles]
    ft [2048, 4096] fp16 : same layout for R[u, item[q0+j]]
  Pad user-rows 8060..8062 carry the bias terms (E row = 1, F row = gb /
  item_bias[item] / user_bias[user]), so the contraction accumulates the
  bias for free.  Phase A input is the transposed fp8 ratings stream
  (item-rows on partitions), 4 blocks packed per row: r8 [2048, 8192].

All reduction matmuls here have M=1, which would waste 127/128 of the PE
array - so they are issued as CONCURRENT col-tiled matmuls
(tile_position=(0, 32j)): phase A packs s_h0/s_h1/c_h0/c_h1 into one PSUM
bank at partitions 0/32/64/96 (4 matmuls run in ~the span of one); phase B
packs p_h0/p_h1 at partitions 0/32.

Per core:
  Phase A (16 macro tiles of 4 blocks):
    s += ones^T @ r8; nnz masks as fp8 0/1 via ACT Sign / DVE is_ne
    (split across engines); c += ones^T @ mask.  All matmuls fp8 N=512.
  AllGather of RAW (s | c) as fp16 (4 KB per core -> 32 KB); the
    masked-mean division happens after the collective, directly in the
    final transposed layout: scT [128, 8, 2, 8] via one rearranged DMA;
    ubTg = s / max(c, 1)  (user 128(8g+t)+p at free index (g, t)).
  Phase B (16 macro tiles): fpt = ft - ub (split ACT bias-activation /
    DVE tensor_scalar), ppt = et * fpt (DVE, some blocks on GpSimd),
    p += ones^T @ ppt (2 col-tiled fp16 matmuls per block, accumulated
    over all 64 blocks).  No prefetch: the r8 stream owns the DMA queues
    first; et/ft stream DMAs are issued after the ub chain and fill the
    queues for the rest of the kernel (pools sized so the stream never
    stalls while the collective is in flight).
  Tail: sigmoid * 5 straight out of PSUM rows 0/32, two DMAs out.
"""

import numpy as np
import ml_dtypes
from contextlib import ExitStack

import concourse.bass as bass
import concourse.bacc as bacc
import concourse.tile as tile
from concourse import mybir
from concourse.bass_utils import run_bass_kernel_spmd

F32 = mybir.dt.float32
F16 = mybir.dt.float16
F8 = mybir.dt.float8e4
NPF16 = np.float16
NPF8 = ml_dtypes.float8_e4m3

NCORES = 8
U = 8001
I = 16001
B = 8192
UPC = 1024          # users per core for phase A (last core: 833 real)
BPC = 1024          # queries per core
MAC = 16            # macro tiles (4 blocks each) in both phases
NBLK = 64           # row blocks of 128 (63 real + 1 pad)
IP2 = 16384         # padded item rows for phase A (64 * 256)
UG2 = 8192          # padded user rows for phase B (64 * 128)

E_BUFS = 7
F_BUFS = 7
R8_BUFS = 12
EARLY_PAIRS = 7
MASK_DVE = {0, 1, 2, 4, 5}   # t % 8 values whose masks run on DVE (rest ACT)
SUB_DVE_MOD = 3     # of every 10 blocks, this many subtracts on DVE (rest ACT)
MULT_GP_MOD = 5     # every MULT_GP_MOD-th block's multiply runs on GpSimd

# bias rows hidden in the user padding (8001..8063)
ROW_GB = 8060
ROW_IB = 8061
ROW_UB = 8062

_CACHED = {}


def build_program():
    nc = bacc.Bacc(num_devices=NCORES, num_swdge_queues=4)

    r8 = nc.dram_tensor("r8", [MAC * 128, 8192], F8, kind="ExternalInput")
    et_d = nc.dram_tensor("et", [MAC * 128, 4096], F16, kind="ExternalInput")
    ft_d = nc.dram_tensor("ft", [MAC * 128, 4096], F16, kind="ExternalInput")
    ident_d = nc.dram_tensor("ident", [128, 128], F16, kind="ExternalInput")
    out = nc.dram_tensor("out", [1, BPC], F32, kind="ExternalOutput")

    with ExitStack() as ctx:
        tc = ctx.enter_context(tile.TileContext(nc))
        singles = ctx.enter_context(tc.tile_pool(name="singles", bufs=1))
        r8_pool = ctx.enter_context(tc.tile_pool(name="r8_pool", bufs=R8_BUFS))
        m_pool = ctx.enter_context(tc.tile_pool(name="m_pool", bufs=6))
        e_pool = ctx.enter_context(tc.tile_pool(name="e_pool", bufs=E_BUFS))
        f_pool = ctx.enter_context(tc.tile_pool(name="f_pool", bufs=F_BUFS))
        fp_pool = ctx.enter_context(tc.tile_pool(name="fp_pool", bufs=6))
        pp_pool = ctx.enter_context(tc.tile_pool(name="pp_pool", bufs=5))
        psA = ctx.enter_context(tc.tile_pool(name="psA", bufs=1, space="PSUM"))
        psB = ctx.enter_context(tc.tile_pool(name="psB", bufs=1, space="PSUM"))
        psC = ctx.enter_context(tc.tile_pool(name="psC", bufs=1, space="PSUM"))
        dram = ctx.enter_context(tc.tile_pool(name="dram", bufs=1, space="DRAM"))

        ones8 = singles.tile([128, 16], F8)
        nc.vector.memset(ones8, 1.0)
        ones16 = singles.tile([128, 1], F16)
        nc.vector.memset(ones16, 1.0)
        ident = singles.tile([128, 128], F16)
        nc.sync.dma_start(ident, ident_d[:, :])

        # ---- Phase A: s/c accumulate in ONE PSUM bank at partitions
        # 0 (s_h0), 32 (s_h1), 64 (c_h0), 96 (c_h1) via col-tiling.
        ps_a = psA.tile([128, 512], F32)

        e_tiles = {}
        f_tiles = {}

        def issue_ef_dma(m, gate=False):
            et = e_pool.tile([128, 4, BPC], F16, name="et_m")
            ft = f_pool.tile([128, 4, BPC], F16, name="ft_m")
            if gate:
                # dummy writes gate the stream DMAs on phase-A progress so
                # the r8 stream keeps DMA priority (WAW dependency)
                nc.vector.memset(et[0:1, 0:1, 0:1], 0.0)
                nc.vector.memset(ft[0:1, 0:1, 0:1], 0.0)
            nc.sync.dma_start(
                et, et_d[m * 128:(m + 1) * 128, :].rearrange(
                    "p (b n) -> p b n", b=4)
            )
            nc.sync.dma_start(
                ft, ft_d[m * 128:(m + 1) * 128, :].rearrange(
                    "p (b n) -> p b n", b=4)
            )
            e_tiles[m] = et
            f_tiles[m] = ft

        for m in range(MAC):
            for b in range(4):
                t = 4 * m + b
                r8t = r8_pool.tile([128, 2, 1024], F8, name="r8t")
                nc.sync.dma_start(
                    r8t, r8[m * 128:(m + 1) * 128,
                            2048 * b:2048 * (b + 1)].rearrange(
                        "p (i n) -> p i n", i=2)
                )
                m8 = m_pool.tile([128, 2, 1024], F8, name="m8")
                if t % 8 in MASK_DVE:
                    nc.vector.tensor_scalar(
                        m8, r8t[:, :, :], 0.0, None,
                        mybir.AluOpType.not_equal,
                    )
                else:
                    nc.scalar.activation(
                        m8, r8t[:, :, :],
                        mybir.ActivationFunctionType.Sign,
                    )
                # 4 concurrent col-tiled matmuls per (block, i): s_h0, s_h1,
                # c_h0, c_h1 at PSUM partitions 0/32/64/96.
                for i in range(2):
                    st = (t == 0 and i == 0)
                    sp = (t == NBLK - 1 and i == 1)
                    for h in range(2):
                        nc.tensor.matmul(
                            ps_a[32 * h:32 * h + 1, :],
                            ones8[:, 0:1],
                            r8t[:, i, h * 512:(h + 1) * 512],
                            start=st, stop=sp,
                            tile_position=(0, 32 * h),
                        )
                        nc.tensor.matmul(
                            ps_a[64 + 32 * h:64 + 32 * h + 1, :],
                            ones8[:, 0:1],
                            m8[:, i, h * 512:(h + 1) * 512],
                            start=st, stop=sp,
                            tile_position=(0, 64 + 32 * h),
                        )
                if b == 3 and m < EARLY_PAIRS:
                    issue_ef_dma(m, gate=True)

        # ---- stage (s | c) to fp16 and AllGather: 4 KB/core -> 32 KB.
        sc_sb = singles.tile([128, 512], F16)
        nc.vector.tensor_copy(sc_sb[0:1, :], ps_a[0:1, :])
        nc.vector.tensor_copy(sc_sb[32:33, :], ps_a[32:33, :])
        nc.scalar.copy(sc_sb[64:65, :], ps_a[64:65, :])
        nc.scalar.copy(sc_sb[96:97, :], ps_a[96:97, :])
        sc_dram = dram.tile([1, 2048], F16, name="sc_dram")
        for j in range(4):
            nc.sync.dma_start(
                sc_dram[0:1, j * 512:(j + 1) * 512], sc_sb[32 * j:32 * j + 1, :]
            )
        cc_out = dram.tile([1, NCORES * 2048], F16, name="cco")
        nc.gpsimd.collective_compute(
            "AllGather",
            mybir.AluOpType.bypass,
            replica_groups=[list(range(NCORES))],
            ins=[sc_dram.opt()],
            outs=[cc_out.opt()],
        )
        # scR[r, c] = cc_out[128r + c]: plain contiguous load (256 B per
        # partition), then one PE transpose -> scT[p, (g h t)] in PSUM.
        scR = singles.tile([128, 128], F16)
        nc.sync.dma_start(
            scR, cc_out[0:1, :].rearrange("o (r c) -> (o r) c", r=128, c=128)
        )
        ps_c = psC.tile([128, 128], F16)
        nc.tensor.transpose(ps_c, scR, ident)
        scT = ps_c[:, :].rearrange("p (g h t) -> p g h t", g=NCORES, h=2, t=8)
        cmaxT = singles.tile([128, NCORES, 8], F32)
        nc.vector.tensor_scalar_max(cmaxT, scT[:, :, 1, :], 1.0)
        crecT = singles.tile([128, NCORES, 8], F32)
        nc.vector.reciprocal(crecT, cmaxT)
        ubTg = singles.tile([128, NCORES, 8], F32)
        nc.vector.tensor_tensor(
            ubTg, scT[:, :, 0, :], crecT, mybir.AluOpType.mult
        )
        negub = singles.tile([128, NCORES, 8], F32)
        nc.vector.tensor_scalar_mul(negub, ubTg, -1.0)

        # ---- phase-B stream DMAs (issued after the ub chain so its small
        # DMAs are not queued behind the 33 MB bulk stream).
        for m in range(EARLY_PAIRS, MAC):
            issue_ef_dma(m)

        # ---- Phase B: p[b] += sum_u et[u, b] * (ft[u, b] - ub[u]).
        # p_h0 accumulates at PSUM partition 0, p_h1 at partition 32.
        # Subtracts split ACT/DVE; multiplies (DVE with a GpSimd share)
        # lag the subtracts by MLAG blocks to avoid head-of-line stalls.
        ps_b = psB.tile([128, 512], F32)
        fpts = {}

        def issue_sub(t):
            m, b = t // 4, t % 4
            g, tt = t // 8, t % 8
            ft = f_tiles[m]
            fpt = fp_pool.tile([128, BPC], F16)
            if t >= NBLK - 4 or t % 12 < 5:
                nc.vector.tensor_scalar(
                    fpt, ft[:, b, :], ubTg[:, g, tt:tt + 1], None,
                    mybir.AluOpType.subtract,
                )
            else:
                nc.scalar.activation(
                    fpt, ft[:, b, :],
                    mybir.ActivationFunctionType.Identity,
                    bias=negub[:, g, tt:tt + 1],
                )
            fpts[t] = fpt

        def issue_mult(t):
            m, b = t // 4, t % 4
            et = e_tiles[m]
            ppt = pp_pool.tile([128, BPC], F16)
            if t % 4 == 2 and t < NBLK - 4:
                nc.gpsimd.tensor_tensor(
                    ppt, et[:, b, :], fpts[t], mybir.AluOpType.mult
                )
            else:
                nc.vector.tensor_tensor(
                    ppt, et[:, b, :], fpts[t], mybir.AluOpType.mult
                )
            del fpts[t]
            for h in range(2):
                nc.tensor.matmul(
                    ps_b[32 * h:32 * h + 1, :],
                    ones16[:, :],
                    ppt[:, h * 512:(h + 1) * 512],
                    start=(t == 0), stop=(t == NBLK - 1),
                    tile_position=(0, 32 * h),
                )

        MLAG = 2
        for t in range(NBLK):
            issue_sub(t)
            if t >= MLAG:
                issue_mult(t - MLAG)
        for t in range(NBLK - MLAG, NBLK):
            issue_mult(t)

        # ---- Tail: sigmoid * 5 straight out of PSUM rows 0 / 32.
        pred = singles.tile([128, 512], F32)
        for h in range(2):
            r = 32 * h
            nc.scalar.activation(
                pred[r:r + 1, :], ps_b[r:r + 1, :],
                mybir.ActivationFunctionType.Sigmoid,
            )
            nc.vector.tensor_scalar_mul(
                pred[r:r + 1, :], pred[r:r + 1, :], 5.0
            )
            nc.sync.dma_start(
                out[0:1, h * 512:(h + 1) * 512], pred[r:r + 1, :]
            )

    nc.finalize()
    return nc


def _pack4(a):
    """[64*128, W] row-blocks -> [16*128, 4*W] macro rows."""
    W = a.shape[1]
    return np.ascontiguousarray(
        a.reshape(MAC, 4, 128, W).transpose(0, 2, 1, 3).reshape(MAC * 128, 4 * W)
    )


def prepare_inputs(user, item, rating_mtx, user_similarity, user_bias,
                   item_bias, global_bias):
    user = np.asarray(user).astype(np.int64)
    item = np.asarray(item).astype(np.int64)
    R = np.asarray(rating_mtx, dtype=np.float32)
    S = np.asarray(user_similarity, dtype=np.float32)
    ubias = np.asarray(user_bias, dtype=np.float32)
    ibias = np.asarray(item_bias, dtype=np.float32)
    gb = np.float32(np.asarray(global_bias))

    R16 = R.astype(NPF16)
    Rt16 = np.ascontiguousarray(R16.T)       # [I, U] for fast column gather
    S16 = S.astype(NPF16)

    in_maps = []
    for k in range(NCORES):
        # ---- phase A: fp8 transposed ratings for this core's user slice,
        # interleaved in (i, p) row pairs per 256-item block.
        u_lo = k * UPC
        u_hi = min(u_lo + UPC, U)
        nu = u_hi - u_lo
        r8full = np.zeros((IP2, UPC), NPF8)
        r8full[:I, :nu] = R[u_lo:u_hi, :].T.astype(NPF8)
        r8i = r8full.reshape(NBLK, 2, 128, UPC).transpose(0, 2, 1, 3).reshape(
            NBLK * 128, 2 * UPC)
        r8 = _pack4(r8i)

        # ---- phase B: dense gathered tables for this core's query slice.
        js = slice(k * BPC, (k + 1) * BPC)
        uk = user[js]
        ik = item[js]
        et = np.zeros((UG2, BPC), NPF16)
        et[:U, :] = np.ascontiguousarray(S16[uk, :].T)
        ft = np.zeros((UG2, BPC), NPF16)
        ft[:U, :] = np.ascontiguousarray(Rt16[ik, :].T)
        # bias terms as extra "users" in the padding
        et[ROW_GB:ROW_UB + 1, :] = NPF16(1.0)
        ft[ROW_GB, :] = NPF16(gb)
        ft[ROW_IB, :] = ibias[ik].astype(NPF16)
        ft[ROW_UB, :] = ubias[uk].astype(NPF16)

        in_maps.append({"r8": r8, "et": _pack4(et), "ft": _pack4(ft),
                        "ident": np.eye(128, dtype=NPF16)})
    return in_maps


def kernel(user, item, rating_mtx, user_similarity, user_bias, item_bias,
           global_bias, _trace=False):
    if "nc" not in _CACHED:
        _CACHED["nc"] = build_program()
    nc = _CACHED["nc"]

    in_maps = prepare_inputs(
        user, item, rating_mtx, user_similarity, user_bias, item_bias,
        global_bias,
    )
    res = run_bass_kernel_spmd(nc, in_maps, core_ids=list(range(NCORES)))
    if _trace:
        # cold traced runs have hung; trace only after a warm run
        res = run_bass_kernel_spmd(
            nc, in_maps, core_ids=list(range(NCORES)), trace=True
        )
    _CACHED["last_results"] = res

    out = np.concatenate(
        [np.asarray(res.results[k]["out"]).reshape(-1) for k in range(NCORES)]
    )
    return out.astype(np.float32)
